# revision 39
# baseline (speedup 1.0000x reference)
"""DeepSeek-MoE block (gate + 2 shared experts + 8 routed experts, top-2)
as a Bass/Tile kernel on 8 Trainium2 NeuronCores.

Sharding (expert-parallel, per the hint):
  - core c owns routed expert c: the host computes the gate (sigmoid
    scores + top-2 + normalized routing weights) in float32 exactly as the
    reference does, and uses it to build the shard map: each core receives
    the *compacted, transposed* token matrix for its expert (the all-to-all
    dispatch), plus its routing weights.  The combine (scatter-add of the
    per-core routed outputs and the sum of the shared-expert partials) is
    the host-side unshard step.
  - the shared expert's FF dim (2816, padded to 3072) is split 384/core, so
    every core produces a partial sum of the shared-expert output.
  - all heavy matmuls run in bf16 (inputs cast host-side, fp32 PSUM
    accumulate): ~4e-3 rel error, far inside the 2e-2 gate, and full PE
    rate.  Routed g/u runs token-stationary (tokens are the PE-stationary
    operand, the g|u-concatenated weights stream as the moving operand in
    512-wide chunks) so every LDWEIGHTS hides under a 512-row matmul.
"""

import numpy as np
from contextlib import ExitStack

import concourse.bass as bass
import concourse.bacc as bacc
import concourse.mybir as mybir
from concourse.tile import TileContext
from concourse import bass_utils

F32 = mybir.dt.float32
BF16 = mybir.dt.bfloat16
AF = mybir.ActivationFunctionType
ALU = mybir.AluOpType

P = 128

# Problem constants (fixed by the graded nn.Module; hardcoded per contract).
HIDDEN = 2048
N_EXPERTS = 8
TOP_K = 2
MOE_FF = 1408
SHARED_FF = 2816
SCALE = 2.5
BATCH, SEQ = 2, 1024
N_CORES = 8

SF_REAL = SHARED_FF // N_CORES      # 352 real shared-FF columns per core
SF = 384                            # padded to a multiple of 128

# Routed-token capacity per expert-core.  The benchmark inputs are
# deterministic (jax.random.key(0)); max tokens/expert is 559.  640 = 5*128.
# kernel() rebuilds with a larger cap if the actual routing ever exceeds it.
CAP = 640


def _fix_matmul_waits(nc):
    """bf16 matmuls lower to an LW+MM pair whose LW struct carries at most
    ONE sync wait; one extra run of the semaphore pass splits multi-waits."""
    import bass_rust as _br
    _br.generate_event_semaphores(nc)


def build_moe_nc(T=BATCH * SEQ, D=HIDDEN, F=MOE_FF, SFp=SF, cap=CAP):
    """Build the SPMD Bass program (same program on all 8 cores)."""
    nc = bacc.Bacc("TRN2", target_bir_lowering=False, debug=False)
    DCH = 512                    # phase-A token chunk (moving free dim)
    NCH = T // DCH
    ND = D // P                  # d blocks (contraction tiles)
    NSJ = SFp // P               # shared f blocks (3)
    NFJ = F // P                 # routed f blocks (11)
    NBC = cap // P               # routed capacity token blocks (5)
    NB = T // P                  # token blocks of 128 (16)
    F2 = 2 * F                   # g|u concatenated routed FF (2816)

    # routed g/u moving chunks over the concatenated g|u axis (<=512 per
    # PSUM bank)
    FCH = []
    o = 0
    while o < F2:
        w = min(512, F2 - o)
        FCH.append((o, w))
        o += w
    NPS = 8                      # PSUM banks rotated through in phase C

    # ---------------- DRAM I/O (all bf16 except the routing weights) ----
    xT = nc.dram_tensor("xT", [D, T], BF16, kind="ExternalInput").ap()
    xeT = nc.dram_tensor("xeT", [D, cap], BF16, kind="ExternalInput").ap()
    swgT = nc.dram_tensor("swgT", [D, SFp], BF16, kind="ExternalInput").ap()
    swuT = nc.dram_tensor("swuT", [D, SFp], BF16, kind="ExternalInput").ap()
    swdT = nc.dram_tensor("swdT", [SFp, D], BF16, kind="ExternalInput").ap()
    wcat = nc.dram_tensor("wcat", [D, F2], BF16, kind="ExternalInput").ap()
    ewdT = nc.dram_tensor("ewdT", [F, D], BF16, kind="ExternalInput").ap()
    gcol = nc.dram_tensor("gcol", [P, NBC], F32, kind="ExternalInput").ap()
    identw = nc.dram_tensor("identw", [P, P], BF16, kind="ExternalInput").ap()

    shared_out = nc.dram_tensor("shared_out", [T, D], BF16,
                                kind="ExternalOutput").ap()
    routed_out = nc.dram_tensor("routed_out", [cap, D], BF16,
                                kind="ExternalOutput").ap()

    with TileContext(nc) as tc, ExitStack() as ctx:
        # ---- long-lived pools ----
        const = ctx.enter_context(tc.tile_pool(name="const", bufs=1))
        ident = const.tile([P, P], BF16, name="ident")
        gcol_sb = const.tile([P, NBC], F32, name="gcol_sb")
        nc.sync.dma_start(gcol_sb, gcol)
        # identity comes from the host: a gpsimd memset/affine_select here
        # would sit in front of chunk 0's activation-stream DMA triggers
        # (the gpsimd queue) and delay the first matmul by ~1.5us

        # resident shared-expert weights (stationary side of phase A);
        # their DMAs are interleaved with chunk 0's activation stream below
        swp = ctx.enter_context(tc.tile_pool(name="sw_res", bufs=1))
        swg_sb = [swp.tile([P, SFp], BF16, name=f"swg{d}", tag=f"swg{d}")
                  for d in range(ND)]
        swu_sb = [swp.tile([P, SFp], BF16, name=f"swu{d}", tag=f"swu{d}")
                  for d in range(ND)]

        shp = ctx.enter_context(tc.tile_pool(name="shT_res", bufs=1))
        shT = [shp.tile([P, T], BF16, name=f"shT{j}", tag=f"shT{j}")
               for j in range(NSJ)]

        # long-lived pools created up-front (pool scopes must nest LIFO);
        # their DMAs are issued later, at the right program points
        xep = ctx.enter_context(tc.tile_pool(name="xe_res", bufs=1))
        xeT_sb = [xep.tile([P, cap], BF16, name=f"xe{d}", tag=f"xe{d}")
                  for d in range(ND)]
        swdp = ctx.enter_context(tc.tile_pool(name="swd_res", bufs=1))
        swd_sb = [swdp.tile([P, D], BF16, name=f"swd{j}", tag=f"swd{j}")
                  for j in range(NSJ)]

        # =========================================================
        # Phase A: shared-expert g/u for all T tokens
        # PSUM: psg/psu x bufs=2 = 4 banks
        # =========================================================
        stmp = ctx.enter_context(tc.tile_pool(name="silu_tmp", bufs=2))
        wstr = ctx.enter_context(tc.tile_pool(name="wstream", bufs=12))
        NPRE = 8
        pre_wt = []
        sA = ExitStack()
        xp = sA.enter_context(tc.tile_pool(name="xT_stream", bufs=2))
        aps = sA.enter_context(tc.tile_pool(name="a_ps", bufs=2, space="PSUM"))

        for ch in range(NCH):
            c0 = ch * DCH
            xt = []
            for d in range(ND):
                t = xp.tile([P, DCH], BF16, name=f"xt{d}", tag=f"xt{d}")
                nc.gpsimd.dma_start(t, xT[d * P:(d + 1) * P, c0:c0 + DCH])
                xt.append(t)
                if ch == 0:
                    # interleave the g-weight loads d-by-d (consumption
                    # order of the first psg d-loop); u-weights follow.
                    # spread the DMA triggers (~600ns each) across three
                    # otherwise-idle sequencers so they don't serialize.
                    nc.sync.dma_start(swg_sb[d], swgT[d * P:(d + 1) * P, :])
                    nc.scalar.dma_start(swu_sb[d], swuT[d * P:(d + 1) * P, :])
            if ch == 1:
                # park the routed-token loads behind chunk 1's stream; they
                # are first needed right after phase A ends
                for d in range(ND):
                    nc.sync.dma_start(xeT_sb[d], xeT[d * P:(d + 1) * P, :])
                nc.scalar.dma_start(ident, identw)
            if ch == 2:
                for j in range(NSJ):
                    nc.sync.dma_start(swd_sb[j], swdT[j * P:(j + 1) * P, :])
            if ch == NCH - 1:
                # prefetch the first routed-weight tiles so phase C's first
                # matmuls don't wait on DMA at the phase boundary
                for d in range(NPRE):
                    wt = wstr.tile([P, 512], BF16, name="wt", tag="wt")
                    nc.scalar.dma_start(wt, wcat[d * P:(d + 1) * P, 0:512])
                    pre_wt.append(wt)

            for j in range(NSJ):
                psg = aps.tile([P, DCH], F32, name="psg", tag="psg")
                psu = aps.tile([P, DCH], F32, name="psu", tag="psu")
                for d in range(ND):
                    nc.tensor.matmul(psg, lhsT=swg_sb[d][:, j * P:(j + 1) * P],
                                     rhs=xt[d],
                                     start=(d == 0), stop=(d == ND - 1))
                for d in range(ND):
                    nc.tensor.matmul(psu, lhsT=swu_sb[d][:, j * P:(j + 1) * P],
                                     rhs=xt[d],
                                     start=(d == 0), stop=(d == ND - 1))
                sgt = stmp.tile([P, DCH], BF16, name="sgt", tag="sgt")
                nc.scalar.activation(sgt, psg, AF.Silu)
                nc.vector.tensor_tensor(shT[j][:, c0:c0 + DCH], sgt, psu,
                                        ALU.mult)
        sA.close()

        # =========================================================
        # Phase C: routed expert g/u, token-stationary.
        # moving operand = g|u-concatenated weights, streamed in 512-chunks;
        # PSUM [128tok, 512] accumulates over d; 8 banks rotate.
        # =========================================================
        hcp = ctx.enter_context(tc.tile_pool(name="hcat", bufs=1))
        hcat = [hcp.tile([P, F2], BF16, name=f"hcat{b}", tag=f"hcat{b}")
                for b in range(NBC)]

        sC = ExitStack()
        rps = sC.enter_context(tc.tile_pool(name="r_ps", bufs=1, space="PSUM"))

        for fc, (o, w) in enumerate(FCH):
            ps = [rps.tile([P, 512], F32, name=f"rp{b}",
                           tag=f"r{(fc * NBC + b) % NPS}")
                  for b in range(NBC)]
            for d in range(ND):
                if fc == 0 and d < NPRE:
                    wt = pre_wt[d]
                else:
                    wt = wstr.tile([P, 512], BF16, name="wt", tag="wt")
                    eng = nc.gpsimd if d % 2 == 0 else nc.sync
                    eng.dma_start(wt[:, :w],
                                  wcat[d * P:(d + 1) * P, o:o + w])
                for b in range(NBC):
                    nc.tensor.matmul(ps[b][:, :w],
                                     lhsT=xeT_sb[d][:, b * P:(b + 1) * P],
                                     rhs=wt[:, :w],
                                     start=(d == 0), stop=(d == ND - 1))
            for b in range(NBC):
                # drain PSUM on scalar+vector (alternating) so neither
                # engine gates PSUM-bank recycling
                if b % 2 == 0:
                    nc.scalar.copy(hcat[b][:, o:o + w], ps[b][:, :w])
                else:
                    nc.vector.tensor_copy(hcat[b][:, o:o + w], ps[b][:, :w])
        sC.close()

        # =========================================================
        # Phase B (shared down-proj) interleaved with Phase D (routed silu +
        # h transposes): B's matmuls keep the PE busy while D's vector work
        # drains; D's transposes slip between B's accumulation groups.
        # PSUM: po0..3 (4 banks) + pt x bufs=2
        # =========================================================
        hTp = ctx.enter_context(tc.tile_pool(name="hT_res", bufs=1))
        hT = [hTp.tile([P, cap], BF16, name=f"hT{j}", tag=f"hT{j}")
              for j in range(NFJ)]

        # routed down-proj weights, fully resident; loaded while B/D runs so
        # phase E never waits on DMA
        HALF = D // 2
        wdp = ctx.enter_context(tc.tile_pool(name="wd_res", bufs=1))
        wd_sb = [[wdp.tile([P, HALF], BF16, name=f"wd{h}_{j}",
                           tag=f"wd{h}_{j}") for j in range(NFJ)]
                 for h in range(2)]
        for h in range(2):
            for j in range(NFJ):
                nc.gpsimd.dma_start(wd_sb[h][j],
                                    ewdT[j * P:(j + 1) * P,
                                         h * HALF:(h + 1) * HALF])

        sBD = ExitStack()
        bps = sBD.enter_context(tc.tile_pool(name="b_ps", bufs=1, space="PSUM"))
        tps = sBD.enter_context(tc.tile_pool(name="t_ps", bufs=2, space="PSUM"))
        sop = sBD.enter_context(tc.tile_pool(name="s_out", bufs=2))
        dtmp = sBD.enter_context(tc.tile_pool(name="d_tmp", bufs=1))
        hsp = sBD.enter_context(tc.tile_pool(name="hs_p", bufs=2))
        NDC = D // 512

        # 16 shared-down token blocks split across the NBC routed blocks
        # (for cap=640 this is [0-3], [4-6], [7-9], [10-12], [13-15])
        tb_groups = [list(g) for g in np.array_split(np.arange(NB), NBC)]

        def shared_down(tb):
            po = [bps.tile([P, 512], F32, name=f"po{k}", tag=f"po{k}")
                  for k in range(NDC)]
            for j in range(NSJ):
                lh = shT[j][:, tb * P:(tb + 1) * P]
                for k in range(NDC):
                    nc.tensor.matmul(po[k], lhsT=lh,
                                     rhs=swd_sb[j][:, k * 512:(k + 1) * 512],
                                     start=(j == 0), stop=(j == NSJ - 1))
            sob = sop.tile([P, D], BF16, name="sob", tag="sob")
            for k in range(NDC):
                # split the PSUM drains across scalar+vector so neither
                # engine gates PSUM-bank recycling (GpSimd can't read PSUM)
                if k < NDC // 2:
                    nc.scalar.copy(sob[:, k * 512:(k + 1) * 512], po[k])
                else:
                    nc.vector.tensor_copy(sob[:, k * 512:(k + 1) * 512], po[k])
            nc.sync.dma_start(shared_out[tb * P:(tb + 1) * P, :], sob)

        eps = sBD.enter_context(tc.tile_pool(name="e_ps", bufs=1, space="PSUM"))
        rop = sBD.enter_context(tc.tile_pool(name="r_out", bufs=2))

        def routed_down(half, b):
            q = [eps.tile([P, 512], F32, name=f"q{k}", tag=f"q{k}")
                 for k in range(HALF // 512)]
            for j in range(NFJ):
                lh = hT[j][:, b * P:(b + 1) * P]
                for k in range(HALF // 512):
                    nc.tensor.matmul(
                        q[k], lhsT=lh,
                        rhs=wd_sb[half][j][:, k * 512:(k + 1) * 512],
                        start=(j == 0), stop=(j == NFJ - 1))
            rob = rop.tile([P, HALF], BF16, name="rob", tag="rob")
            for k in range(HALF // 512):
                nc.scalar.copy(rob[:, k * 512:(k + 1) * 512], q[k])
                nc.sync.dma_start(
                    routed_out[b * P:(b + 1) * P,
                               half * HALF + k * 512:
                               half * HALF + (k + 1) * 512],
                    rob[:, k * 512:(k + 1) * 512])

        for b in range(NBC):
            # D: silu(g)*u*gate_weight for routed block b (scalar/vector)
            sg = dtmp.tile([P, F], BF16, name="sg", tag="sg")
            nc.scalar.activation(sg, hcat[b][:, :F], AF.Silu)
            t3 = dtmp.tile([P, F], BF16, name="t3", tag="t3")
            nc.vector.tensor_tensor(t3, sg, hcat[b][:, F:], ALU.mult)
            hs = hsp.tile([P, F], BF16, name="hs", tag="hs")
            nc.vector.tensor_scalar(hs, t3, gcol_sb[:, b:b + 1], None,
                                    op0=ALU.mult)
            # B: shared down-proj chunk (fills the PE meanwhile)
            for tb in tb_groups[b]:
                shared_down(tb)
            # D: transpose h block b into [f, tok] for the down-proj
            for j in range(NFJ):
                pt = tps.tile([P, P], BF16, name="pt", tag="pt")
                nc.tensor.transpose(pt, hs[:, j * P:(j + 1) * P], ident)
                nc.vector.tensor_copy(hT[j][:, b * P:(b + 1) * P], pt)
            # E: routed down-proj for block b, interleaved right behind its
            # transposes so the PE never drains at the phase boundary
            routed_down(0, b)
            routed_down(1, b)
        sBD.close()


    nc.compile()
    _fix_matmul_waits(nc)
    return nc


# ---------------------------------------------------------------------------
# Host orchestration: gate + dispatch (the shard map) and combine (unshard)
# ---------------------------------------------------------------------------

_NC_CACHE = {}


def _get_nc(cap):
    if cap not in _NC_CACHE:
        _NC_CACHE[cap] = build_moe_nc(cap=cap)
    return _NC_CACHE[cap]


def _bf16(a):
    import ml_dtypes
    return np.ascontiguousarray(np.asarray(a, np.float32)).astype(
        ml_dtypes.bfloat16)


def _dispatch(x2, gate_w):
    """Float32 gate, exactly the reference computation."""
    logits = x2 @ np.asarray(gate_w, np.float32).T          # [T, E]
    scores = 1.0 / (1.0 + np.exp(-logits))
    idx = np.argpartition(-scores, TOP_K, axis=1)[:, :TOP_K]  # top-2 set
    vals = np.take_along_axis(scores, idx, 1)
    w = vals / (vals.sum(1, keepdims=True) + 1e-20) * SCALE
    return idx, w


def _shard_inputs(hidden_states, gate_w, shared_wg, shared_wu, shared_wd,
                  exp_wg, exp_wu, exp_wd, cap):
    T, D = BATCH * SEQ, HIDDEN
    f32 = np.float32
    x2 = np.asarray(hidden_states, f32).reshape(T, D)
    idx, w = _dispatch(x2, gate_w)

    xT_b = _bf16(x2.T)
    swgT_full = np.asarray(shared_wg, f32).T    # [D, SHARED_FF]
    swuT_full = np.asarray(shared_wu, f32).T
    swdT_full = np.asarray(shared_wd, f32).T    # [SHARED_FF, D]

    in_maps, sels = [], []
    for c in range(N_CORES):
        m = (idx == c)
        sel = np.nonzero(m.any(1))[0]
        n_c = len(sel)
        assert n_c <= cap, f"expert {c} got {n_c} tokens > cap {cap}"
        wc = np.where(m[sel, 0], w[sel, 0], w[sel, 1]).astype(f32)

        xe = np.zeros((cap, D), f32)
        xe[:n_c] = x2[sel]
        gc = np.zeros(cap, f32)
        gc[:n_c] = wc

        sl = slice(c * SF_REAL, (c + 1) * SF_REAL)
        swgT_c = np.zeros((D, SF), f32)
        swgT_c[:, :SF_REAL] = swgT_full[:, sl]
        swuT_c = np.zeros((D, SF), f32)
        swuT_c[:, :SF_REAL] = swuT_full[:, sl]
        swdT_c = np.zeros((SF, D), f32)
        swdT_c[:SF_REAL, :] = swdT_full[sl, :]

        wcat_c = np.concatenate(
            [np.asarray(exp_wg[c], f32).T, np.asarray(exp_wu[c], f32).T],
            axis=1)                                          # [D, 2F]

        in_maps.append({
            "identw": _bf16(np.eye(P, dtype=f32)),
            "xT": xT_b,
            "xeT": _bf16(xe.T),
            "swgT": _bf16(swgT_c),
            "swuT": _bf16(swuT_c),
            "swdT": _bf16(swdT_c),
            "wcat": _bf16(wcat_c),
            "ewdT": _bf16(np.asarray(exp_wd[c], f32).T),
            "gcol": np.ascontiguousarray(
                gc.reshape(cap // P, P).T).astype(f32),
        })
        sels.append(sel)
    return in_maps, sels


def _combine(results, sels):
    T, D = BATCH * SEQ, HIDDEN
    out = np.zeros((T, D), np.float32)
    for r, sel in zip(results, sels):
        out += np.asarray(r["shared_out"], np.float32)
        np.add.at(out, sel,
                  np.asarray(r["routed_out"][:len(sel)], np.float32))
    return out.reshape(BATCH, SEQ, HIDDEN)


def _required_cap(hidden_states, gate_w):
    x2 = np.asarray(hidden_states, np.float32).reshape(BATCH * SEQ, HIDDEN)
    idx, _ = _dispatch(x2, gate_w)
    n_max = int(np.bincount(idx.ravel(), minlength=N_EXPERTS).max())
    return max(CAP, -(-n_max // P) * P)


def kernel(**inputs):
    cap = _required_cap(inputs["hidden_states"], inputs["gate_w"])
    nc = _get_nc(cap)
    in_maps, sels = _shard_inputs(**inputs, cap=cap)
    res = bass_utils.run_bass_kernel_spmd(nc, in_maps,
                                          core_ids=list(range(N_CORES)))
    return _combine(res.results, sels)


def run_traced(trace_cores=None, **inputs):
    """test-only entry: returns (output, BassKernelResults with exec time)."""
    cap = _required_cap(inputs["hidden_states"], inputs["gate_w"])
    nc = _get_nc(cap)
    in_maps, sels = _shard_inputs(**inputs, cap=cap)
    kw = {}
    if trace_cores is not None:
        kw["trace_cores"] = trace_cores
    res = bass_utils.run_bass_kernel_spmd(
        nc, in_maps, core_ids=list(range(N_CORES)), trace=True, **kw)
    return _combine(res.results, sels), res



# revision 44
# speedup vs baseline: 1.0001x; 1.0001x over previous
"""DeepSeek-MoE block (gate + 2 shared experts + 8 routed experts, top-2)
as a Bass/Tile kernel on 8 Trainium2 NeuronCores.

Sharding (expert-parallel, per the hint):
  - core c owns routed expert c: the host computes the gate (sigmoid
    scores + top-2 + normalized routing weights) in float32 exactly as the
    reference does, and uses it to build the shard map: each core receives
    the *compacted, transposed* token matrix for its expert (the all-to-all
    dispatch), plus its routing weights.  The combine (scatter-add of the
    per-core routed outputs and the sum of the shared-expert partials) is
    the host-side unshard step.
  - the shared expert's FF dim (2816, padded to 3072) is split 384/core, so
    every core produces a partial sum of the shared-expert output.
  - all heavy matmuls run in bf16 (inputs cast host-side, fp32 PSUM
    accumulate): ~4e-3 rel error, far inside the 2e-2 gate, and full PE
    rate.  Routed g/u runs token-stationary (tokens are the PE-stationary
    operand, the g|u-concatenated weights stream as the moving operand in
    512-wide chunks) so every LDWEIGHTS hides under a 512-row matmul.
"""

import numpy as np
from contextlib import ExitStack

import concourse.bass as bass
import concourse.bacc as bacc
import concourse.mybir as mybir
from concourse.tile import TileContext
from concourse.masks import make_identity
from concourse import bass_utils

F32 = mybir.dt.float32
BF16 = mybir.dt.bfloat16
AF = mybir.ActivationFunctionType
ALU = mybir.AluOpType

P = 128

# Problem constants (fixed by the graded nn.Module; hardcoded per contract).
HIDDEN = 2048
N_EXPERTS = 8
TOP_K = 2
MOE_FF = 1408
SHARED_FF = 2816
SCALE = 2.5
BATCH, SEQ = 2, 1024
N_CORES = 8

SF_REAL = SHARED_FF // N_CORES      # 352 real shared-FF columns per core
SF = 384                            # padded to a multiple of 128

# Routed-token capacity per expert-core.  The benchmark inputs are
# deterministic (jax.random.key(0)); max tokens/expert is 559.  640 = 5*128.
# kernel() rebuilds with a larger cap if the actual routing ever exceeds it.
CAP = 640


def _fix_matmul_waits(nc):
    """bf16 matmuls lower to an LW+MM pair whose LW struct carries at most
    ONE sync wait; one extra run of the semaphore pass splits multi-waits."""
    import bass_rust as _br
    _br.generate_event_semaphores(nc)


def build_moe_nc(T=BATCH * SEQ, D=HIDDEN, F=MOE_FF, SFp=SF, cap=CAP):
    """Build the SPMD Bass program (same program on all 8 cores)."""
    nc = bacc.Bacc("TRN2", target_bir_lowering=False, debug=False)
    DCH = 512                    # phase-A token chunk (moving free dim)
    NCH = T // DCH
    ND = D // P                  # d blocks (contraction tiles)
    NSJ = SFp // P               # shared f blocks (3)
    NFJ = F // P                 # routed f blocks (11)
    NBC = cap // P               # routed capacity token blocks (5)
    NB = T // P                  # token blocks of 128 (16)
    F2 = 2 * F                   # g|u concatenated routed FF (2816)

    # routed g/u moving chunks over the concatenated g|u axis (<=512 per
    # PSUM bank)
    FCH = []
    o = 0
    while o < F2:
        w = min(512, F2 - o)
        FCH.append((o, w))
        o += w
    NPS = 8                      # PSUM banks rotated through in phase C

    # ---------------- DRAM I/O (all bf16 except the routing weights) ----
    xT = nc.dram_tensor("xT", [D, T], BF16, kind="ExternalInput").ap()
    xeT = nc.dram_tensor("xeT", [D, cap], BF16, kind="ExternalInput").ap()
    swgT = nc.dram_tensor("swgT", [D, SFp], BF16, kind="ExternalInput").ap()
    swuT = nc.dram_tensor("swuT", [D, SFp], BF16, kind="ExternalInput").ap()
    swdT = nc.dram_tensor("swdT", [SFp, D], BF16, kind="ExternalInput").ap()
    wcat = nc.dram_tensor("wcat", [D, F2], BF16, kind="ExternalInput").ap()
    ewdT = nc.dram_tensor("ewdT", [F, D], BF16, kind="ExternalInput").ap()
    gcol = nc.dram_tensor("gcol", [P, NBC], F32, kind="ExternalInput").ap()

    shared_out = nc.dram_tensor("shared_out", [T, D], BF16,
                                kind="ExternalOutput").ap()
    routed_out = nc.dram_tensor("routed_out", [cap, D], BF16,
                                kind="ExternalOutput").ap()

    with TileContext(nc) as tc, ExitStack() as ctx:
        # ---- long-lived pools ----
        const = ctx.enter_context(tc.tile_pool(name="const", bufs=1))
        ident = const.tile([P, P], BF16, name="ident")
        make_identity(nc, ident)
        gcol_sb = const.tile([P, NBC], F32, name="gcol_sb")
        nc.sync.dma_start(gcol_sb, gcol)

        # resident shared-expert weights (stationary side of phase A);
        # their DMAs are interleaved with chunk 0's activation stream below
        swp = ctx.enter_context(tc.tile_pool(name="sw_res", bufs=1))
        swg_sb = [swp.tile([P, SFp], BF16, name=f"swg{d}", tag=f"swg{d}")
                  for d in range(ND)]
        swu_sb = [swp.tile([P, SFp], BF16, name=f"swu{d}", tag=f"swu{d}")
                  for d in range(ND)]

        shp = ctx.enter_context(tc.tile_pool(name="shT_res", bufs=1))
        shT = [shp.tile([P, T], BF16, name=f"shT{j}", tag=f"shT{j}")
               for j in range(NSJ)]

        # long-lived pools created up-front (pool scopes must nest LIFO);
        # their DMAs are issued later, at the right program points
        xep = ctx.enter_context(tc.tile_pool(name="xe_res", bufs=1))
        xeT_sb = [xep.tile([P, cap], BF16, name=f"xe{d}", tag=f"xe{d}")
                  for d in range(ND)]
        swdp = ctx.enter_context(tc.tile_pool(name="swd_res", bufs=1))
        swd_sb = [swdp.tile([P, D], BF16, name=f"swd{j}", tag=f"swd{j}")
                  for j in range(NSJ)]

        # =========================================================
        # Phase A: shared-expert g/u for all T tokens
        # PSUM: psg/psu x bufs=2 = 4 banks
        # =========================================================
        stmp = ctx.enter_context(tc.tile_pool(name="silu_tmp", bufs=2))
        wstr = ctx.enter_context(tc.tile_pool(name="wstream", bufs=12))
        NPRE = 8
        pre_wt = []
        sA = ExitStack()
        xp = sA.enter_context(tc.tile_pool(name="xT_stream", bufs=2))
        aps = sA.enter_context(tc.tile_pool(name="a_ps", bufs=2, space="PSUM"))

        for ch in range(NCH):
            c0 = ch * DCH
            xt = []
            for d in range(ND):
                t = xp.tile([P, DCH], BF16, name=f"xt{d}", tag=f"xt{d}")
                nc.gpsimd.dma_start(t, xT[d * P:(d + 1) * P, c0:c0 + DCH])
                xt.append(t)
                if ch == 0:
                    # interleave the g-weight loads d-by-d (consumption
                    # order of the first psg d-loop); u-weights follow.
                    # spread the DMA triggers (~600ns each) across three
                    # otherwise-idle sequencers so they don't serialize.
                    nc.sync.dma_start(swg_sb[d], swgT[d * P:(d + 1) * P, :])
                    nc.scalar.dma_start(swu_sb[d], swuT[d * P:(d + 1) * P, :])
            if ch == 1:
                # park the routed-token loads behind chunk 1's stream; they
                # are first needed right after phase A ends
                for d in range(ND):
                    nc.sync.dma_start(xeT_sb[d], xeT[d * P:(d + 1) * P, :])
            if ch == 2:
                for j in range(NSJ):
                    nc.sync.dma_start(swd_sb[j], swdT[j * P:(j + 1) * P, :])
            if ch == NCH - 1:
                # prefetch the first routed-weight tiles so phase C's first
                # matmuls don't wait on DMA at the phase boundary
                for d in range(NPRE):
                    wt = wstr.tile([P, 512], BF16, name="wt", tag="wt")
                    nc.scalar.dma_start(wt, wcat[d * P:(d + 1) * P, 0:512])
                    pre_wt.append(wt)

            for j in range(NSJ):
                psg = aps.tile([P, DCH], F32, name="psg", tag="psg")
                psu = aps.tile([P, DCH], F32, name="psu", tag="psu")
                for d in range(ND):
                    nc.tensor.matmul(psg, lhsT=swg_sb[d][:, j * P:(j + 1) * P],
                                     rhs=xt[d],
                                     start=(d == 0), stop=(d == ND - 1))
                for d in range(ND):
                    nc.tensor.matmul(psu, lhsT=swu_sb[d][:, j * P:(j + 1) * P],
                                     rhs=xt[d],
                                     start=(d == 0), stop=(d == ND - 1))
                sgt = stmp.tile([P, DCH], BF16, name="sgt", tag="sgt")
                nc.scalar.activation(sgt, psg, AF.Silu)
                nc.vector.tensor_tensor(shT[j][:, c0:c0 + DCH], sgt, psu,
                                        ALU.mult)
        sA.close()

        # =========================================================
        # Phase C: routed expert g/u, token-stationary.
        # moving operand = g|u-concatenated weights, streamed in 512-chunks;
        # PSUM [128tok, 512] accumulates over d; 8 banks rotate.
        # =========================================================
        hcp = ctx.enter_context(tc.tile_pool(name="hcat", bufs=1))
        hcat = [hcp.tile([P, F2], BF16, name=f"hcat{b}", tag=f"hcat{b}")
                for b in range(NBC)]

        sC = ExitStack()
        rps = sC.enter_context(tc.tile_pool(name="r_ps", bufs=1, space="PSUM"))

        for fc, (o, w) in enumerate(FCH):
            ps = [rps.tile([P, 512], F32, name=f"rp{b}",
                           tag=f"r{(fc * NBC + b) % NPS}")
                  for b in range(NBC)]
            for d in range(ND):
                if fc == 0 and d < NPRE:
                    wt = pre_wt[d]
                else:
                    wt = wstr.tile([P, 512], BF16, name="wt", tag="wt")
                    eng = nc.gpsimd if d % 2 == 0 else nc.sync
                    eng.dma_start(wt[:, :w],
                                  wcat[d * P:(d + 1) * P, o:o + w])
                for b in range(NBC):
                    nc.tensor.matmul(ps[b][:, :w],
                                     lhsT=xeT_sb[d][:, b * P:(b + 1) * P],
                                     rhs=wt[:, :w],
                                     start=(d == 0), stop=(d == ND - 1))
            for b in range(NBC):
                # drain PSUM on scalar+vector (alternating) so neither
                # engine gates PSUM-bank recycling
                if b % 2 == 0:
                    nc.scalar.copy(hcat[b][:, o:o + w], ps[b][:, :w])
                else:
                    nc.vector.tensor_copy(hcat[b][:, o:o + w], ps[b][:, :w])
        sC.close()

        # =========================================================
        # Phase B (shared down-proj) interleaved with Phase D (routed silu +
        # h transposes): B's matmuls keep the PE busy while D's vector work
        # drains; D's transposes slip between B's accumulation groups.
        # PSUM: po0..3 (4 banks) + pt x bufs=2
        # =========================================================
        hTp = ctx.enter_context(tc.tile_pool(name="hT_res", bufs=1))
        hT = [hTp.tile([P, cap], BF16, name=f"hT{j}", tag=f"hT{j}")
              for j in range(NFJ)]

        # routed down-proj weights, fully resident; loaded while B/D runs so
        # phase E never waits on DMA
        HALF = D // 2
        wdp = ctx.enter_context(tc.tile_pool(name="wd_res", bufs=1))
        wd_sb = [[wdp.tile([P, HALF], BF16, name=f"wd{h}_{j}",
                           tag=f"wd{h}_{j}") for j in range(NFJ)]
                 for h in range(2)]
        for h in range(2):
            for j in range(NFJ):
                nc.gpsimd.dma_start(wd_sb[h][j],
                                    ewdT[j * P:(j + 1) * P,
                                         h * HALF:(h + 1) * HALF])

        sBD = ExitStack()
        bps = sBD.enter_context(tc.tile_pool(name="b_ps", bufs=1, space="PSUM"))
        tps = sBD.enter_context(tc.tile_pool(name="t_ps", bufs=2, space="PSUM"))
        sop = sBD.enter_context(tc.tile_pool(name="s_out", bufs=2))
        dtmp = sBD.enter_context(tc.tile_pool(name="d_tmp", bufs=1))
        hsp = sBD.enter_context(tc.tile_pool(name="hs_p", bufs=2))
        NDC = D // 512

        # 16 shared-down token blocks split across the NBC routed blocks
        # (for cap=640 this is [0-3], [4-6], [7-9], [10-12], [13-15])
        tb_groups = [list(g) for g in np.array_split(np.arange(NB), NBC)]

        def shared_down(tb):
            po = [bps.tile([P, 512], F32, name=f"po{k}", tag=f"po{k}")
                  for k in range(NDC)]
            for j in range(NSJ):
                lh = shT[j][:, tb * P:(tb + 1) * P]
                for k in range(NDC):
                    nc.tensor.matmul(po[k], lhsT=lh,
                                     rhs=swd_sb[j][:, k * 512:(k + 1) * 512],
                                     start=(j == 0), stop=(j == NSJ - 1))
            sob = sop.tile([P, D], BF16, name="sob", tag="sob")
            for k in range(NDC):
                # split the PSUM drains across scalar+vector so neither
                # engine gates PSUM-bank recycling (GpSimd can't read PSUM)
                if k < NDC // 2:
                    nc.scalar.copy(sob[:, k * 512:(k + 1) * 512], po[k])
                else:
                    nc.vector.tensor_copy(sob[:, k * 512:(k + 1) * 512], po[k])
            nc.sync.dma_start(shared_out[tb * P:(tb + 1) * P, :], sob)

        eps = sBD.enter_context(tc.tile_pool(name="e_ps", bufs=1, space="PSUM"))
        rop = sBD.enter_context(tc.tile_pool(name="r_out", bufs=2))

        def routed_down(half, b):
            q = [eps.tile([P, 512], F32, name=f"q{k}", tag=f"q{k}")
                 for k in range(HALF // 512)]
            for j in range(NFJ):
                lh = hT[j][:, b * P:(b + 1) * P]
                for k in range(HALF // 512):
                    nc.tensor.matmul(
                        q[k], lhsT=lh,
                        rhs=wd_sb[half][j][:, k * 512:(k + 1) * 512],
                        start=(j == 0), stop=(j == NFJ - 1))
            rob = rop.tile([P, HALF], BF16, name="rob", tag="rob")
            for k in range(HALF // 512):
                nc.scalar.copy(rob[:, k * 512:(k + 1) * 512], q[k])
                nc.sync.dma_start(
                    routed_out[b * P:(b + 1) * P,
                               half * HALF + k * 512:
                               half * HALF + (k + 1) * 512],
                    rob[:, k * 512:(k + 1) * 512])

        for b in range(NBC):
            # D: silu(g)*u*gate_weight for routed block b (scalar/vector)
            sg = dtmp.tile([P, F], BF16, name="sg", tag="sg")
            nc.scalar.activation(sg, hcat[b][:, :F], AF.Silu)
            t3 = dtmp.tile([P, F], BF16, name="t3", tag="t3")
            nc.vector.tensor_tensor(t3, sg, hcat[b][:, F:], ALU.mult)
            hs = hsp.tile([P, F], BF16, name="hs", tag="hs")
            nc.vector.tensor_scalar(hs, t3, gcol_sb[:, b:b + 1], None,
                                    op0=ALU.mult)
            # B: shared down-proj chunk (fills the PE meanwhile)
            for tb in tb_groups[b]:
                shared_down(tb)
            # D: transpose h block b into [f, tok] for the down-proj
            for j in range(NFJ):
                pt = tps.tile([P, P], BF16, name="pt", tag="pt")
                nc.tensor.transpose(pt, hs[:, j * P:(j + 1) * P], ident)
                nc.vector.tensor_copy(hT[j][:, b * P:(b + 1) * P], pt)
            # E: routed down-proj for block b, interleaved right behind its
            # transposes so the PE never drains at the phase boundary
            routed_down(0, b)
            routed_down(1, b)
        sBD.close()


    nc.compile()
    _fix_matmul_waits(nc)
    return nc


# ---------------------------------------------------------------------------
# Host orchestration: gate + dispatch (the shard map) and combine (unshard)
# ---------------------------------------------------------------------------

_NC_CACHE = {}


def _get_nc(cap):
    if cap not in _NC_CACHE:
        _NC_CACHE[cap] = build_moe_nc(cap=cap)
    return _NC_CACHE[cap]


def _bf16(a):
    import ml_dtypes
    return np.ascontiguousarray(np.asarray(a, np.float32)).astype(
        ml_dtypes.bfloat16)


def _dispatch(x2, gate_w):
    """Float32 gate, exactly the reference computation."""
    logits = x2 @ np.asarray(gate_w, np.float32).T          # [T, E]
    scores = 1.0 / (1.0 + np.exp(-logits))
    idx = np.argpartition(-scores, TOP_K, axis=1)[:, :TOP_K]  # top-2 set
    vals = np.take_along_axis(scores, idx, 1)
    w = vals / (vals.sum(1, keepdims=True) + 1e-20) * SCALE
    return idx, w


def _shard_inputs(hidden_states, gate_w, shared_wg, shared_wu, shared_wd,
                  exp_wg, exp_wu, exp_wd, cap):
    T, D = BATCH * SEQ, HIDDEN
    f32 = np.float32
    x2 = np.asarray(hidden_states, f32).reshape(T, D)
    idx, w = _dispatch(x2, gate_w)

    xT_b = _bf16(x2.T)
    swgT_full = np.asarray(shared_wg, f32).T    # [D, SHARED_FF]
    swuT_full = np.asarray(shared_wu, f32).T
    swdT_full = np.asarray(shared_wd, f32).T    # [SHARED_FF, D]

    in_maps, sels = [], []
    for c in range(N_CORES):
        m = (idx == c)
        sel = np.nonzero(m.any(1))[0]
        n_c = len(sel)
        assert n_c <= cap, f"expert {c} got {n_c} tokens > cap {cap}"
        wc = np.where(m[sel, 0], w[sel, 0], w[sel, 1]).astype(f32)

        xe = np.zeros((cap, D), f32)
        xe[:n_c] = x2[sel]
        gc = np.zeros(cap, f32)
        gc[:n_c] = wc

        sl = slice(c * SF_REAL, (c + 1) * SF_REAL)
        swgT_c = np.zeros((D, SF), f32)
        swgT_c[:, :SF_REAL] = swgT_full[:, sl]
        swuT_c = np.zeros((D, SF), f32)
        swuT_c[:, :SF_REAL] = swuT_full[:, sl]
        swdT_c = np.zeros((SF, D), f32)
        swdT_c[:SF_REAL, :] = swdT_full[sl, :]

        wcat_c = np.concatenate(
            [np.asarray(exp_wg[c], f32).T, np.asarray(exp_wu[c], f32).T],
            axis=1)                                          # [D, 2F]

        in_maps.append({
            "xT": xT_b,
            "xeT": _bf16(xe.T),
            "swgT": _bf16(swgT_c),
            "swuT": _bf16(swuT_c),
            "swdT": _bf16(swdT_c),
            "wcat": _bf16(wcat_c),
            "ewdT": _bf16(np.asarray(exp_wd[c], f32).T),
            "gcol": np.ascontiguousarray(
                gc.reshape(cap // P, P).T).astype(f32),
        })
        sels.append(sel)
    return in_maps, sels


def _combine(results, sels):
    T, D = BATCH * SEQ, HIDDEN
    out = np.zeros((T, D), np.float32)
    for r, sel in zip(results, sels):
        out += np.asarray(r["shared_out"], np.float32)
        np.add.at(out, sel,
                  np.asarray(r["routed_out"][:len(sel)], np.float32))
    return out.reshape(BATCH, SEQ, HIDDEN)


def _required_cap(hidden_states, gate_w):
    x2 = np.asarray(hidden_states, np.float32).reshape(BATCH * SEQ, HIDDEN)
    idx, _ = _dispatch(x2, gate_w)
    n_max = int(np.bincount(idx.ravel(), minlength=N_EXPERTS).max())
    return max(CAP, -(-n_max // P) * P)


def kernel(**inputs):
    cap = _required_cap(inputs["hidden_states"], inputs["gate_w"])
    nc = _get_nc(cap)
    in_maps, sels = _shard_inputs(**inputs, cap=cap)
    res = bass_utils.run_bass_kernel_spmd(nc, in_maps,
                                          core_ids=list(range(N_CORES)))
    return _combine(res.results, sels)


def run_traced(trace_cores=None, **inputs):
    """test-only entry: returns (output, BassKernelResults with exec time)."""
    cap = _required_cap(inputs["hidden_states"], inputs["gate_w"])
    nc = _get_nc(cap)
    in_maps, sels = _shard_inputs(**inputs, cap=cap)
    kw = {}
    if trace_cores is not None:
        kw["trace_cores"] = trace_cores
    res = bass_utils.run_bass_kernel_spmd(
        nc, in_maps, core_ids=list(range(N_CORES)), trace=True, **kw)
    return _combine(res.results, sels), res

